# revision 6
# baseline (speedup 1.0000x reference)
"""Trainium2 Bass kernel for nn_AdaptiveKernelModule (dense_cnn).

Math: the per-sample dynamic conv kernel is rank-2 in its output channel:
    gk[o,i,kh,kw] = Wk[o] * g[i,kh,kw] + bk[o]
so with u = Wf@Wk, v = Wf@bk, w = Wf@b_adap + bf (host-precomputed):
    out[c, p] = u[c] * A[p] + v[c] * B[p] + w[c]
    A[p] = sum_{i,kh,kw} g[i,kh,kw] * f[i, p + delta(kh,kw)]
    B[p] = sum_{i,kh,kw}              f[i, p + delta(kh,kw)]
    f    = relu(W1 @ x + b1)

Device pipeline per sample (2 samples per core, 8 cores data-parallel over N):
  MM1: f_psum = W1Tx4.T @ x_chunk   (K=128, M=128 replicated x4 for fp32r),
       ACT relu+b1 on the c0 block -> f_pad (SBUF, zero border)
  maxpool 64x64 windows on DVE -> xp[128,10]; tiny MM + relu -> g[32,9]
  MM2: T_psum = G2.T @ f_pad_chunk  (K=32, M=128; cols 96..104 = g taps,
       col 105 = ones, rest zero), DVE copy rows 96..105 -> T_plain
  DMA SBUF->SBUF: T_sb[t, q] = T_plain[row(t), q + delta_t]  (18 shifted rows)
  MM3: out_psum = L3.T @ T_sb_chunk  (K=18, M=128), L3 = [u]*9 + [v]*9,
       ACT Identity+bias(w) -> out tile -> DMA to HBM.
All matmuls float32r (fp32 HIGH mode, 1 cyc/row at N>=256, col_grp=0xf).
fp32r operands must be produced with f32r-typed output APs (verifier rule);
memset cannot emit f32r (ISA), so zero/one fills go through DVE broadcast
copies from f32 scratch tiles.
"""

import numpy as np

import concourse.bass as bass
import concourse.bacc as bacc
import concourse.mybir as mybir
import concourse.tile as tile
from concourse.bass_utils import run_bass_kernel_spmd

F32 = mybir.dt.float32
F32R = mybir.dt.float32r

N_CORES = 8
NS = 2            # samples per core
C = 128           # input channels
CM = 32           # bottleneck channels
H = W = 192
HP = WP = 194     # padded
L = HP * WP       # padded pixels per plane (37636)
XROWS = 8         # image rows per x/out tile
RROWS = 2         # image rows per matmul chunk (N = 2*192 = 384)

# partition layout inside the mega tile
FP0, FP1 = 0, 32      # f_pad for sample buffer 0 / 1 (32 partitions each)
TSB = 64              # T_sb: 18 shifted tap rows
TPL = 96              # T_plain: 10 raw rows (9 g-taps + 1 ones-tap)
GCOL = 96             # G2 column where tap columns start (96..104 taps, 105 ones)

DELTAS = [(kh - 1) * WP + (kw - 1) for kh in range(3) for kw in range(3)]


def build(nc):
    x_d = nc.declare_dram_parameter("x", [NS, C, H, W], F32, isOutput=False)
    w1t_d = nc.declare_dram_parameter("w1t", [C, C], F32, isOutput=False)
    b1_d = nc.declare_dram_parameter("b1", [CM, 1], F32, isOutput=False)
    l3_d = nc.declare_dram_parameter("l3", [18, C], F32, isOutput=False)
    wb_d = nc.declare_dram_parameter("wb", [C, 1], F32, isOutput=False)
    out_d = nc.declare_dram_parameter("out", [NS, C, H, W], F32, isOutput=True)

    with tile.TileContext(nc) as tc:
        with (
            tc.tile_pool(name="persist", bufs=1) as pp,
            tc.tile_pool(name="xin", bufs=3) as xin_pool,
            tc.tile_pool(name="outp", bufs=2) as out_pool,
            tc.tile_pool(name="small", bufs=2) as sp,
            tc.tile_pool(name="psum", bufs=2, space="PSUM") as psp,
        ):
            mega = pp.tile([128, L], F32)
            w1t_sb = pp.tile([C, C], F32)
            b1_sb = pp.tile([64, 1], F32)
            l3_sb = pp.tile([128, C], F32)
            wb_sb = pp.tile([C, 1], F32)
            g2 = pp.tile([64, C], F32)
            zsc = pp.tile([64, 1], F32)
            osc = pp.tile([64, 1], F32)

            nc.sync.dma_start(
                out=w1t_sb[:, :].bitcast(F32R), in_=w1t_d.ap().bitcast(F32R)
            )
            nc.sync.dma_start(out=b1_sb[0:32, :], in_=b1_d.ap())
            nc.sync.dma_start(out=b1_sb[32:64, :], in_=b1_d.ap())
            nc.sync.dma_start(
                out=l3_sb[64:82, :].bitcast(F32R), in_=l3_d.ap().bitcast(F32R)
            )
            nc.sync.dma_start(out=wb_sb[:, :], in_=wb_d.ap())

            nc.vector.memset(zsc[:, :], 0.0)
            nc.vector.memset(osc[:, :], 1.0)

            # G2 stationary operand: zero everywhere, ones in col GCOL+9;
            # per-sample g taps land in cols GCOL..GCOL+8 via the ACT evac.
            nc.vector.tensor_copy(
                g2[:, :].bitcast(F32R), zsc[:, :].broadcast_to([64, C])
            )
            nc.vector.tensor_copy(g2[:, GCOL + 9 : GCOL + 10].bitcast(F32R), osc[:, :])

            # zero the f_pad borders for both sample buffers (never overwritten)
            meg3 = mega[0:64, :].rearrange("p (r c) -> p r c", c=WP)
            nc.vector.tensor_copy(
                mega[0:64, 0:WP].bitcast(F32R), zsc[:, :].broadcast_to([64, WP])
            )
            nc.vector.tensor_copy(
                mega[0:64, (HP - 1) * WP : HP * WP].bitcast(F32R),
                zsc[:, :].broadcast_to([64, WP]),
            )
            nc.vector.tensor_copy(
                meg3[:, :, 0:1].squeeze(axis=2).bitcast(F32R),
                zsc[:, :].broadcast_to([64, HP]),
            )
            nc.vector.tensor_copy(
                meg3[:, :, WP - 1 : WP].squeeze(axis=2).bitcast(F32R),
                zsc[:, :].broadcast_to([64, HP]),
            )

            for n in range(NS):
                c0 = FP0 if n % 2 == 0 else FP1
                fpad = mega[c0 : c0 + CM, :]
                fpad3 = fpad.rearrange("p (r c) -> p r c", c=WP)
                b1n = b1_sb[c0 : c0 + CM, :]

                # ---------------- pass 1: x in, maxpool partials, MM1+relu
                xp_part = sp.tile([128, 72], F32, tag="xp_part")
                ntiles = H // XROWS  # 24
                for j in range(ntiles):
                    xt = xin_pool.tile([128, XROWS * W], F32, tag="xt")
                    xt3 = xt.rearrange("p (r c) -> p r c", c=W)
                    nc.sync.dma_start(
                        out=xt3.bitcast(F32R),
                        in_=x_d.ap()[n, :, j * XROWS : (j + 1) * XROWS, :].bitcast(
                            F32R
                        ),
                    )
                    # maxpool partial over this 8-row band: out [128, 3]
                    xt4 = xt.rearrange("p (r kx c) -> p kx r c", kx=3, c=64)
                    nc.vector.tensor_reduce(
                        xp_part[:, 3 * j : 3 * j + 3],
                        xt4,
                        axis=mybir.AxisListType.XY,
                        op=mybir.AluOpType.max,
                    )
                    for r in range(XROWS // RROWS):
                        y0 = j * XROWS + r * RROWS
                        pf = psp.tile([128, 512], F32, tag="pf", name="pf")[
                            :, : RROWS * W
                        ]
                        nc.tensor.matmul(
                            pf[:, :],
                            w1t_sb[:, :].bitcast(F32R),
                            xt[:, r * RROWS * W : (r + 1) * RROWS * W].bitcast(F32R),
                        )
                        nc.scalar.activation(
                            fpad3[:, y0 + 1 : y0 + 1 + RROWS, 1 : 1 + W].bitcast(F32R),
                            pf[c0 : c0 + CM, :].rearrange("p (r c) -> p r c", c=W),
                            mybir.ActivationFunctionType.Relu,
                            bias=b1n,
                        )

                # ---------------- finalize maxpool, compute g
                xp_f = sp.tile([128, 10], F32, tag="xp_f")
                xp_r = sp.tile([128, 10], F32, tag="xp_r")
                nc.vector.memset(xp_f[:, 9:10], 0.0)
                nc.vector.tensor_reduce(
                    xp_f[:, 0:9],
                    xp_part.rearrange("p (ky s kx) -> p ky kx s", ky=3, kx=3),
                    axis=mybir.AxisListType.X,
                    op=mybir.AluOpType.max,
                )
                nc.vector.tensor_copy(xp_r[:, :].bitcast(F32R), xp_f[:, :])
                pg = psp.tile([128, 512], F32, tag="pg", name="pg")[:, :10]
                nc.tensor.matmul(
                    pg[:, :],
                    w1t_sb[:, :].bitcast(F32R),
                    xp_r[:, :].bitcast(F32R),
                )
                nc.scalar.activation(
                    g2[c0 : c0 + CM, GCOL : GCOL + 9].bitcast(F32R),
                    pg[c0 : c0 + CM, 0:9],
                    mybir.ActivationFunctionType.Relu,
                    bias=b1n,
                )

                # ---------------- MM2: T = G2.T @ f_pad, all padded rows
                tpl = mega[TPL : TPL + 10, :]
                for p0 in range(0, HP, RROWS):
                    pT = psp.tile([128, 512], F32, tag="pT", name="pT")[
                        :, : RROWS * WP
                    ]
                    nc.tensor.matmul(
                        pT[:, :],
                        g2[c0 : c0 + CM, :].bitcast(F32R),
                        fpad[:, p0 * WP : (p0 + RROWS) * WP].bitcast(F32R),
                    )
                    nc.vector.tensor_copy(
                        tpl[:, p0 * WP : (p0 + RROWS) * WP].bitcast(F32R),
                        pT[TPL : TPL + 10, :],
                    )

                # ---------------- shifted tap copies SBUF->SBUF
                NSPLIT = 2
                for t in range(18):
                    src = TPL + (t if t < 9 else 9)
                    d = DELTAS[t % 9]
                    a = max(0, -d)
                    b = L - max(0, d)
                    step = (b - a + NSPLIT - 1) // NSPLIT
                    for s in range(NSPLIT):
                        lo = a + s * step
                        hi = min(b, lo + step)
                        nc.sync.dma_start(
                            out=mega[TSB + t : TSB + t + 1, lo:hi].bitcast(F32R),
                            in_=mega[src : src + 1, lo + d : hi + d].bitcast(F32R),
                        )

                # ---------------- MM3 + bias + store
                tsb = mega[TSB : TSB + 18, :].rearrange("p (r c) -> p r c", c=WP)
                for j in range(ntiles):
                    ot = out_pool.tile([128, XROWS * W], F32, tag="ot")
                    for r in range(XROWS // RROWS):
                        y0 = j * XROWS + r * RROWS
                        po = psp.tile([128, 512], F32, tag="po", name="po")[
                            :, : RROWS * W
                        ]
                        nc.tensor.matmul(
                            po[:, :],
                            l3_sb[64:82, :].bitcast(F32R),
                            tsb[:, y0 + 1 : y0 + 1 + RROWS, 1 : 1 + W].bitcast(F32R),
                        )
                        nc.scalar.activation(
                            ot[:, r * RROWS * W : (r + 1) * RROWS * W],
                            po[:, :],
                            mybir.ActivationFunctionType.Identity,
                            bias=wb_sb[:, :],
                        )
                    nc.sync.dma_start(
                        out=out_d.ap()[n, :, j * XROWS : (j + 1) * XROWS, :],
                        in_=ot.rearrange("p (r c) -> p r c", c=W),
                    )
    return nc


_CACHE = {}


def _get_nc():
    if "nc" not in _CACHE:
        nc = bacc.Bacc(
            "TRN2", target_bir_lowering=False, debug=False, num_devices=N_CORES
        )
        build(nc)
        nc.compile()
        _CACHE["nc"] = nc
    return _CACHE["nc"]


def make_in_maps(x, W1, b1, Wk, bk, b_adap, Wf, bf):
    x = np.asarray(x, dtype=np.float32)
    W1 = np.asarray(W1, dtype=np.float32)
    b1 = np.asarray(b1, dtype=np.float32)
    Wk = np.asarray(Wk, dtype=np.float32)
    bk = np.asarray(bk, dtype=np.float32)
    b_adap = np.asarray(b_adap, dtype=np.float32)
    Wf = np.asarray(Wf, dtype=np.float32)
    bf = np.asarray(bf, dtype=np.float32)

    u = Wf @ Wk                # [128]
    v = Wf @ bk                # [128]
    w = Wf @ b_adap + bf       # [128]
    l3 = np.ascontiguousarray(np.stack([u] * 9 + [v] * 9).astype(np.float32))
    w1t = np.ascontiguousarray(np.tile(W1.T, (1, 4)).astype(np.float32))
    b1c = np.ascontiguousarray(b1[:, None].astype(np.float32))
    wbc = np.ascontiguousarray(w[:, None].astype(np.float32))

    in_maps = []
    for i in range(N_CORES):
        in_maps.append(
            {
                "x": np.ascontiguousarray(x[i * NS : (i + 1) * NS]),
                "w1t": w1t,
                "b1": b1c,
                "l3": l3,
                "wb": wbc,
            }
        )
    return in_maps


def kernel(x, W1, b1, Wk, bk, b_adap, Wf, bf):
    nc = _get_nc()
    in_maps = make_in_maps(x, W1, b1, Wk, bk, b_adap, Wf, bf)
    res = run_bass_kernel_spmd(nc, in_maps, list(range(N_CORES)))
    return np.concatenate([res.results[i]["out"] for i in range(N_CORES)], axis=0)
